# revision 29
# baseline (speedup 1.0000x reference)
"""Trainium2 Bass kernel for nn_Attention (llama-style attention block, GQA, RoPE).

Distribution (8 NeuronCores, Megatron-style tensor parallel over heads):
  - Each core gets 4 Q heads + its matching 1 KV head (wq/wk/wv output-dim sharded).
  - Attention computed per-core in a transposed dataflow (head_dim on partitions,
    tokens on the free dim) so every matmul streams 512-wide moving operands
    at full float32r rate and no probability transposes are needed.
  - Two per-batch AllToAlls reshard the attention output token-parallel (each
    overlaps remaining compute); each core then runs the wo projection for its
    2x256-token block against the full wo (contract over all 4096 head dims),
    so no AllReduce is needed. Host reassembles the per-core blocks.

float32r rules (walrus BIR verifier): a matmul input buffer must be produced
with an f32r-typed output AP. DMA (with both APs bitcast to f32r) and ACT
activations can produce f32r; DVE ops cannot, so masking is applied additively
to the scores PSUM before the exp.
"""

import sys

if "/opt/trn_rl_repo" not in sys.path:
    sys.path.insert(0, "/opt/trn_rl_repo")

import numpy as np

N_CORES = 8
B, S, D = 2, 2048, 4096
N_HEADS = 32
N_KV_HEADS = 8
HEAD_DIM = 128
H_PER_CORE = N_HEADS // N_CORES          # 4 q heads per core
TOK = B * S                              # 4096 flattened tokens
QKV_M = H_PER_CORE * HEAD_DIM + 2 * HEAD_DIM  # 768 projection rows per core
PROJ_TOK = 512                           # token block in the projection stage
SQ_BLK = 512                             # sq block in attention
A2A_TOK = SQ_BLK // 2                    # tokens per rank per per-batch AllToAll
N_SQ_BLK = S // SQ_BLK                   # 4 per batch
N_TCHUNK = S // HEAD_DIM                 # 16 key chunks per batch
SCALE = 1.0 / float(np.sqrt(HEAD_DIM))
NKC = D // 128                           # 32 contraction chunks

# partition permutation for RoPE: pair (even, odd) lives 16 partitions apart
# inside a 32-partition quadrant, so the rotation is a single stream_shuffle.
_P = np.arange(128)
_I_OF_P = 16 * (_P // 32) + (_P % 32) % 16          # rope pair index 0..63
_IS_ODD = (_P % 32) >= 16
PERM = (2 * _I_OF_P + _IS_ODD.astype(np.int64)).astype(np.int64)  # orig row in head block
SHUF_MASK = [(i + 16) % 32 for i in range(32)]

_PROGRAMS = {}


def _build_program(mask_mode):
    """Build + compile the SPMD program. mask_mode in {'causal', 'none', 'general'}."""
    import concourse.bass as bass
    import concourse.mybir as mybir
    import concourse.tile as tile
    from concourse import bacc
    from concourse.masks import make_identity

    f32 = mybir.dt.float32
    f32r = mybir.dt.float32r
    Exp = mybir.ActivationFunctionType.Exp

    nc = bacc.Bacc("TRN2", target_bir_lowering=False, debug=False,
                   num_devices=N_CORES)

    xT = nc.dram_tensor("xT", [D, TOK], f32, kind="ExternalInput")
    wqkvT = nc.dram_tensor("wqkvT", [D, QKV_M], f32, kind="ExternalInput")
    # wo pre-tiled on host: [m_chunk, p, k_chunk, m_col] so each stationary
    # column-block DMA reads 16KB-contiguous lines
    woT4 = nc.dram_tensor("woT4", [NKC, 128, NKC, 128], f32, kind="ExternalInput")
    cos2 = nc.dram_tensor("cos2", [128, S], f32, kind="ExternalInput")
    sin2 = nc.dram_tensor("sin2", [128, S], f32, kind="ExternalInput")
    if mask_mode == "general":
        # additive mask stored transposed: maskT[k_pos, q_pos]
        maskT = nc.dram_tensor("maskT", [S, S], f32, kind="ExternalInput")
    out_d = nc.dram_tensor("out", [D, SQ_BLK], f32, kind="ExternalOutput")

    def r(ap):
        return ap.bitcast(f32r)

    xT_t = xT.ap().rearrange("(k p) t -> p k t", p=128)      # [128, 32, TOK]
    wqkvT_t = wqkvT.ap().rearrange("(k p) m -> p k m", p=128)  # [128, 32, 768]

    with tile.TileContext(nc) as tc:
        with tc.tile_pool(name="const", bufs=1) as const, \
             tc.tile_pool(name="dram", bufs=1, space="DRAM") as dram:
            # per-core q/k/v (transposed), split per batch so attention on
            # batch 0 only depends on the first half of the projection
            qd = [dram.tile([H_PER_CORE * 128, S], f32, name=f"qd{b_}")
                  for b_ in range(B)]
            kd = [dram.tile([128, S], f32, name=f"kd{b_}") for b_ in range(B)]
            vd = [dram.tile([128, S], f32, name=f"vd{b_}") for b_ in range(B)]
            a2a_in = [dram.tile([N_CORES, SQ_BLK + H_PER_CORE, A2A_TOK], f32,
                                name=f"a2a_in{b_}") for b_ in range(B)]
            a2a_out = [dram.tile([N_CORES, SQ_BLK + H_PER_CORE, A2A_TOK], f32,
                                 name=f"a2a_out{b_}") for b_ in range(B)]

            # f32r constants must be ACT-produced (DVE cannot write f32r)
            ones_f32 = const.tile([128, 128], f32)
            nc.vector.memset(ones_f32[:], 1.0)
            ones_col = const.tile([128, 1], f32r)     # lhsT for column sums
            nc.scalar.copy(ones_col[:], ones_f32[:, 0:1])
            ones_row = const.tile([1, 128], f32r)     # lhsT for partition broadcast
            nc.scalar.copy(ones_row[:], ones_f32[0:1, :])
            ident = const.tile([128, 128], f32)       # fp32 PE transpose identity
            make_identity(nc, ident[:])
            # selector stationaries for the post-a2a sum broadcast:
            # sel[c, kc, p] = (c == kc), so matmul(sel[:,kc,:], rsum[32,t])
            # replicates rsum row kc across all 128 output partitions
            if mask_mode == "causal":
                # additive causal mask pairs: [pi][cols 0:512]=shift 2*pi,
                # [cols 512:1024]=shift 2*pi+1; 0.0 where f >= p + 128*shift
                cmask = const.tile([128, 2, 2 * SQ_BLK], f32)
                nc.gpsimd.memset(cmask[:], 0.0)
                for pi in range(2):
                    for half in range(2):
                        sh = 2 * pi + half
                        nc.gpsimd.affine_select(
                            out=cmask[:, pi, half * SQ_BLK:(half + 1) * SQ_BLK],
                            in_=cmask[:, pi, half * SQ_BLK:(half + 1) * SQ_BLK],
                            pattern=[[1, SQ_BLK]], base=-128 * sh,
                            channel_multiplier=-1,
                            compare_op=mybir.AluOpType.is_ge, fill=-1e9,
                        )

            # ---------------- stage 1: fused QKV projection + RoPE ----------------
            # k-outer / m-inner with 6 live PSUM accumulation groups, so the
            # two half-K x tiles (xA, xB) double-buffer against each other.
            n_blk = TOK // PROJ_TOK
            HK = NKC // 2
            with tc.tile_pool(name="pj_w", bufs=1) as pj_w, \
                 tc.tile_pool(name="pj_x", bufs=1) as pj_x, \
                 tc.tile_pool(name="pj_cs", bufs=2) as pj_cs, \
                 tc.tile_pool(name="pj_t", bufs=2) as pj_t, \
                 tc.tile_pool(name="pj_o", bufs=2) as pj_o, \
                 tc.tile_pool(name="pj_ps", bufs=8, space="PSUM") as pj_ps:
                w_sb = pj_w.tile([128, NKC, QKV_M], f32r)
                x0 = slice(0, PROJ_TOK)
                xA0 = pj_x.tile([128, HK, PROJ_TOK], f32r, tag="xA")
                xB0 = pj_x.tile([128, HK, PROJ_TOK], f32r, tag="xB")
                nc.sync.dma_start(xA0[:], r(xT_t[:, 0:HK, x0]))
                # split the weight load by k-chunk so the first matmuls can
                # start before the full 12.6MB arrives
                for kw in range(0, NKC, 4):
                    nc.sync.dma_start(w_sb[:, kw:kw + 4, :],
                                      r(wqkvT_t[:, kw:kw + 4, :]))
                    if kw == 0:
                        nc.sync.dma_start(xB0[:], r(xT_t[:, HK:NKC, x0]))
                for n in range(n_blk):
                    s0 = (n * PROJ_TOK) % S  # position within the batch
                    bn = n // (S // PROJ_TOK)  # batch of this token block
                    cols = slice(n * PROJ_TOK, (n + 1) * PROJ_TOK)
                    bcols = slice(s0, s0 + PROJ_TOK)
                    if n == 0:
                        xA, xB = xA0, xB0
                    else:
                        xA = pj_x.tile([128, HK, PROJ_TOK], f32r, tag="xA")
                        xB = pj_x.tile([128, HK, PROJ_TOK], f32r, tag="xB")
                        nc.sync.dma_start(xA[:], r(xT_t[:, 0:HK, cols]))
                        nc.sync.dma_start(xB[:], r(xT_t[:, HK:NKC, cols]))
                    c_sb = pj_cs.tile([128, PROJ_TOK], f32, tag="c")
                    s_sb = pj_cs.tile([128, PROJ_TOK], f32, tag="s")
                    nc.sync.dma_start(c_sb[:], cos2.ap()[:, s0:s0 + PROJ_TOK])
                    nc.sync.dma_start(s_sb[:], sin2.ap()[:, s0:s0 + PROJ_TOK])
                    pss = [pj_ps.tile([128, PROJ_TOK], f32, tag="ps",
                                      name=f"ps_{n}_{mi}")
                           for mi in range(QKV_M // 128)]
                    for k in range(NKC):
                        xsb = xA if k < HK else xB
                        xi = k if k < HK else k - HK
                        for m in range(QKV_M // 128):
                            nc.tensor.matmul(
                                pss[m][:], w_sb[:, k, m * 128:(m + 1) * 128],
                                xsb[:, xi, :],
                                start=(k == 0), stop=(k == NKC - 1))
                    for m in range(QKV_M // 128):  # q0..q3, k, v
                        ps = pss[m]
                        o_sb = pj_o.tile([128, PROJ_TOK], f32, tag="o")
                        if m < 5:  # rope for q heads + k
                            tmp = pj_t.tile([128, PROJ_TOK], f32, tag="tmp")
                            rot = pj_t.tile([128, PROJ_TOK], f32, tag="rot")
                            t1 = pj_t.tile([128, PROJ_TOK], f32, tag="t1")
                            nc.scalar.copy(tmp[:], ps[:])
                            nc.vector.stream_shuffle(rot[:], tmp[:], SHUF_MASK)
                            nc.vector.tensor_mul(t1[:], tmp[:], c_sb[:])
                            nc.vector.tensor_mul(rot[:], rot[:], s_sb[:])
                            nc.vector.tensor_add(o_sb[:], t1[:], rot[:])
                        else:
                            nc.scalar.copy(o_sb[:], ps[:])
                        if m < 4:
                            dst = qd[bn][m * 128:(m + 1) * 128, bcols]
                        elif m == 4:
                            dst = kd[bn][:, bcols]
                        else:
                            dst = vd[bn][:, bcols]
                        nc.sync.dma_start(dst, o_sb[:])

            # ---------------- stage 2: attention + per-batch AllToAll ----------------
            with tc.tile_pool(name="at_kv", bufs=2) as at_kv, \
                 tc.tile_pool(name="at_q", bufs=2) as at_q, \
                 tc.tile_pool(name="at_e", bufs=4) as at_e, \
                 tc.tile_pool(name="at_o", bufs=2) as at_o, \
                 tc.tile_pool(name="at_sm", bufs=4) as at_sm, \
                 tc.tile_pool(name="ps_s", bufs=2, space="PSUM") as ps_s, \
                 tc.tile_pool(name="ps_av", bufs=2, space="PSUM") as ps_av, \
                 tc.tile_pool(name="ps_sum", bufs=1, space="PSUM") as ps_sum, \
                 tc.tile_pool(name="ps_misc", bufs=1, space="PSUM") as ps_misc:
                for b in range(B):
                    kT = at_kv.tile([128, S], f32r, tag="kT")
                    vT = at_kv.tile([128, S], f32, tag="vT")
                    nc.sync.dma_start(kT[:], r(kd[b][:]))
                    nc.sync.dma_start(vT[:], vd[b][:])
                    v_nat = at_kv.tile([128, N_TCHUNK, 128], f32r, tag="vn")
                    for i in range(N_TCHUNK):
                        tp = ps_misc.tile([128, 128], f32, tag="misc")
                        nc.tensor.transpose(
                            tp[:], vT[:, i * 128:(i + 1) * 128], ident[:])
                        nc.scalar.copy(v_nat[:, i, :], tp[:])
                    for h in range(H_PER_CORE):
                        qT = at_q.tile([128, S], f32r, tag="qT")
                        nc.sync.dma_start(
                            qT[:], r(qd[b][h * 128:(h + 1) * 128, :]))
                        for j in range(N_SQ_BLK):
                            npair = 2 * j + 2 if mask_mode == "causal" else N_TCHUNK // 2
                            qs = qT[:, j * SQ_BLK:(j + 1) * SQ_BLK]
                            av = ps_av.tile([128, SQ_BLK], f32, tag="av")
                            sm = ps_sum.tile([1, SQ_BLK], f32, tag="sum")
                            for p_ in range(npair):
                                i0, i1 = 2 * p_, 2 * p_ + 1
                                sp = ps_s.tile([128, 2 * SQ_BLK], f32, tag="s")
                                nc.tensor.matmul(
                                    sp[:, 0:SQ_BLK], kT[:, i0 * 128:(i0 + 1) * 128],
                                    qs, start=True, stop=True)
                                nc.tensor.matmul(
                                    sp[:, SQ_BLK:], kT[:, i1 * 128:(i1 + 1) * 128],
                                    qs, start=True, stop=True)
                                if mask_mode == "causal" and p_ >= 2 * j:
                                    nc.vector.tensor_add(
                                        sp[:], sp[:], cmask[:, p_ - 2 * j, :])
                                elif mask_mode == "general":
                                    mt = at_e.tile([128, 2, SQ_BLK], f32, tag="mt")
                                    nc.sync.dma_start(
                                        mt[:],
                                        maskT.ap()[i0 * 128:(i0 + 2) * 128,
                                                   j * SQ_BLK:(j + 1) * SQ_BLK]
                                        .rearrange("(c p) q -> p c q", p=128))
                                    nc.vector.tensor_add(
                                        sp[:], sp[:],
                                        mt[:].rearrange("p c q -> p (c q)"))
                                e = at_e.tile([128, 2 * SQ_BLK], f32r, tag="e")
                                nc.scalar.activation(e[:], sp[:], Exp, scale=SCALE)
                                last = (p_ == npair - 1)
                                nc.tensor.matmul(
                                    av[:], v_nat[:, i0, :], e[:, 0:SQ_BLK],
                                    start=(p_ == 0), stop=False)
                                nc.tensor.matmul(
                                    av[:], v_nat[:, i1, :], e[:, SQ_BLK:],
                                    start=False, stop=last)
                                nc.tensor.matmul(
                                    sm[:], ones_col[:], e[:, 0:SQ_BLK],
                                    start=(p_ == 0), stop=False)
                                nc.tensor.matmul(
                                    sm[:], ones_col[:], e[:, SQ_BLK:],
                                    start=False, stop=last)
                            ssb = at_sm.tile([1, SQ_BLK], f32, tag="ssb")
                            nc.vector.tensor_copy(ssb[:], sm[:])
                            at = at_o.tile([128, SQ_BLK], f32, tag="at")
                            nc.vector.tensor_copy(at[:], av[:])
                            # tokens [512j, 512j+512) of batch b span a2a
                            # blocks 2j and 2j+1; sums ride along in rows
                            # 512+h of each block
                            nc.sync.dma_start(
                                a2a_in[b][2 * j:2 * j + 2,
                                          h * 128:(h + 1) * 128, :]
                                .rearrange("jb p t -> p jb t"),
                                at[:].rearrange("p (jb t) -> p jb t", jb=2))
                            nc.sync.dma_start(
                                a2a_in[b][2 * j:2 * j + 2,
                                          SQ_BLK + h:SQ_BLK + h + 1, :]
                                .rearrange("jb one t -> one jb t"),
                                ssb[:].rearrange("p (jb t) -> p jb t", jb=2))
                    nc.gpsimd.collective_compute(
                        "AllToAll", mybir.AluOpType.bypass,
                        replica_groups=[list(range(N_CORES))],
                        ins=[a2a_in[b].opt()], outs=[a2a_out[b].opt()],
                    )

            # ---------------- stage 3: wo projection (2x256 owned tokens) ----------
            with tc.tile_pool(name="wo_a", bufs=1) as wo_a, \
                 tc.tile_pool(name="wo_n", bufs=4) as wo_n, \
                 tc.tile_pool(name="wo_w", bufs=3) as wo_w, \
                 tc.tile_pool(name="wo_o", bufs=3) as wo_o, \
                 tc.tile_pool(name="wo_ps", bufs=4, space="PSUM") as wo_ps:
                a_sb = wo_a.tile([128, NKC, SQ_BLK], f32r)
                for b in range(B):
                    araw = wo_a.tile([128, NKC, A2A_TOK], f32, tag="araw",
                                     name=f"araw{b}")
                    for rr_ in range(N_CORES):
                        nc.sync.dma_start(
                            araw[:, 4 * rr_:4 * (rr_ + 1), :],
                            a2a_out[b][rr_, 0:SQ_BLK, :]
                            .rearrange("(k2 p) t -> p k2 t", p=128))
                    for kcp in range(NKC // 2):
                        # sums for head-chunk pair (2kcp, 2kcp+1) = rank
                        # kcp//2, heads (2kcp)%4 and +1 -> [1, 512] on part 0
                        rr = kcp // 2
                        h0 = (2 * kcp) % H_PER_CORE
                        ssb2 = wo_n.tile([1, 2 * A2A_TOK], f32, tag="ssb2")
                        nc.sync.dma_start(
                            ssb2[:].rearrange("one (c t) -> one c t", c=2),
                            a2a_out[b][rr, SQ_BLK + h0:SQ_BLK + h0 + 2, :]
                            .unsqueeze(0))
                        rs2 = wo_n.tile([1, 2 * A2A_TOK], f32, tag="rs2")
                        nc.vector.reciprocal_approx_fast(out=rs2[:], in_=ssb2[:])
                        rs2_r = wo_n.tile([1, 2 * A2A_TOK], f32r, tag="rs2_r")
                        nc.scalar.copy(rs2_r[:], rs2[:])
                        bc = wo_ps.tile([128, 2 * A2A_TOK], f32, tag="bc")
                        nc.tensor.matmul(bc[:], ones_row[:], rs2_r[:],
                                         start=True, stop=True)
                        for sub in range(2):
                            kc = 2 * kcp + sub
                            nrm = wo_n.tile([128, A2A_TOK], f32, tag="nrm")
                            nc.vector.tensor_mul(
                                nrm[:], araw[:, kc, :],
                                bc[:, sub * A2A_TOK:(sub + 1) * A2A_TOK])
                            nc.scalar.copy(
                                a_sb[:, kc, b * A2A_TOK:(b + 1) * A2A_TOK],
                                nrm[:])
                for m in range(NKC):
                    w_sb2 = wo_w.tile([128, NKC, 128], f32r, tag="w")
                    nc.sync.dma_start(w_sb2[:], r(woT4.ap()[m]))
                    ps = wo_ps.tile([128, SQ_BLK], f32, tag="ps")
                    for k in range(NKC):
                        nc.tensor.matmul(
                            ps[:], w_sb2[:, k, :], a_sb[:, k, :],
                            start=(k == 0), stop=(k == NKC - 1))
                    o_sb = wo_o.tile([128, SQ_BLK], f32, tag="o")
                    nc.vector.tensor_copy(o_sb[:], ps[:])
                    nc.sync.dma_start(
                        out_d.ap()[m * 128:(m + 1) * 128, :], o_sb[:])

    nc.compile()
    return nc


def _get_program(mask_mode):
    if mask_mode not in _PROGRAMS:
        _PROGRAMS[mask_mode] = _build_program(mask_mode)
    return _PROGRAMS[mask_mode]


def _classify_mask(m2):
    if not m2.any():
        return "none"
    causal_ref = np.triu(np.full((S, S), -1e9, dtype=np.float32), k=1)
    return "causal" if np.array_equal(m2, causal_ref) else "general"


def _prep_inputs(x, freqs_cos, freqs_sin, mask, wq, wk, wv, wo):
    """Host-side sharding / layout prep shared by kernel() and test.py."""
    m2 = np.asarray(mask, np.float32).reshape(S, S)
    mask_mode = _classify_mask(m2)

    xT = np.ascontiguousarray(np.asarray(x, np.float32).reshape(TOK, D).T)
    woT = np.asarray(wo, np.float32).T          # [hd_in, D_out]
    # pre-tile wo for contiguous stationary-block DMAs:
    # woT4[m, p, k, mcol] = woT[k*128+p, m*128+mcol]
    woT4 = np.ascontiguousarray(
        woT.reshape(NKC, 128, NKC, 128).transpose(2, 1, 0, 3))

    fc = np.asarray(freqs_cos, np.float32)
    fs = np.asarray(freqs_sin, np.float32)
    cos2 = np.ascontiguousarray(fc.T[_I_OF_P, :])            # [128, S]
    sgn = np.where(_IS_ODD, 1.0, -1.0).astype(np.float32)[:, None]
    sin2 = np.ascontiguousarray(fs.T[_I_OF_P, :] * sgn)

    def permute_heads(w):
        w4 = np.asarray(w, np.float32).reshape(-1, HEAD_DIM, D)
        return w4[:, PERM, :].reshape(-1, D)

    wq_p = permute_heads(wq)
    wk_p = permute_heads(wk)
    wv = np.asarray(wv, np.float32)

    in_maps = []
    for c in range(N_CORES):
        wqkvT = np.ascontiguousarray(np.concatenate(
            [wq_p[c * 512:(c + 1) * 512], wk_p[c * 128:(c + 1) * 128],
             wv[c * 128:(c + 1) * 128]], axis=0).T)           # [D, 768]
        m = {"xT": xT, "wqkvT": wqkvT, "woT4": woT4, "cos2": cos2, "sin2": sin2}
        if mask_mode == "general":
            m["maskT"] = np.ascontiguousarray(m2.T)
        in_maps.append(m)
    return mask_mode, in_maps


def kernel(x, start_pos, freqs_cos, freqs_sin, mask, cache_k, cache_v,
           wq, wk, wv, wo):
    from concourse.bass_utils import run_bass_kernel_spmd

    assert int(start_pos) == 0, "kernel compiled for start_pos == 0"
    mask_mode, in_maps = _prep_inputs(x, freqs_cos, freqs_sin, mask,
                                      wq, wk, wv, wo)
    nc = _get_program(mask_mode)
    res = run_bass_kernel_spmd(nc, in_maps, list(range(N_CORES)))
    out = np.empty((TOK, D), dtype=np.float32)
    for c in range(N_CORES):
        blk = res.results[c]["out"]                  # [D, 512]
        for b in range(B):
            rows = slice(b * S + A2A_TOK * c, b * S + A2A_TOK * (c + 1))
            out[rows, :] = blk[:, b * A2A_TOK:(b + 1) * A2A_TOK].T
    return out.reshape(B, S, D)
